# revision 6
# baseline (speedup 1.0000x reference)
"""LIF neuron (STBP) forward kernel for Trainium2, 8-core data parallel.

Reference semantics (per element, scan over T):
    v = v * 0.9 + x_t
    s = (v >= 1.0)
    v = v - s * 1.0

Sharding: batch dim 32 -> 8 cores x 4; the recurrence is elementwise per
neuron so cores are independent.

Layout: per core the input is relayouted on host to partition-major
[P=128, T*2048] f32; the whole 16-MiB input lives in one SBUF arena.
The kernel is HBM-load-bound (16 MiB of f32 input at ~350 GB/s/core),
so the design keeps every engine's busy time under the ~48 us load
window and shrinks store traffic 8x by bit-packing spikes on chip:

  Vector (DVE):   ONLY the serial LIF recurrence, one fused custom op
                  per step:  u' = (u - (u >= th)) * beta + x
  Scalar (Act):   spike extraction  s_t = sign(u_t - 1) in {-1,0,+1}
                  as bf16 (plus the two final pack copies psum->u8)
  Tensor (PE):    bit packing: psum += diag(2^(t%8)) @ s_t, so after 8
                  steps psum = sum_j +-2^j = 2*bits - 255; the Act copy
                  0.5*psum + 127.5 -> u8 yields the packed spike byte
  Sync/Act HWDGE: the 16 input loads (t14/t15 split small so the last
                  bytes land early); the 4 tail u8 stores
  GpSimd SWDGE:   the mid-kernel plane-A store (hidden under loads)

Host decodes spikes with unpackbits (bit j of plane g = spike at
t = 8g + j); sign(0)=0 at the measure-zero tie u == 1 is absorbed by
the rel-err budget.
"""

from contextlib import ExitStack

import numpy as np

import concourse.bacc as bacc
import concourse.mybir as mybir
import concourse.tile as tile
from concourse.bass_utils import run_bass_kernel_spmd

N_CORES = 8
B, T, C, H, W = 32, 16, 64, 32, 32
B_LOC = B // N_CORES  # 4 batches per core
P = 128               # SBUF partitions
F = (C * H * W) // P  # 512 free elements per partition per batch
FB = B_LOC * F        # 2048 free elements in a fused all-batch tile
Q = FB // 4           # 512 = max matmul moving free dim
BETA = 0.9
V_TH = 1.0

_CACHE = {}


def _get_lif_op():
    """Register (once) and return the fused LIF membrane-update DVE op."""
    import concourse.dve_ops as dve_ops
    from concourse.dve_ops import DveOp
    from concourse.dve_spec import C0, C1, Spec, Src0, Src1

    for o in dve_ops.OPS:
        if o.name == "LIF_U_ANT":
            return o

    op = DveOp(
        "LIF_U_ANT",
        Spec(
            body=(Src0 - (Src0 >= C1)) * C0 + Src1,
            reference=lambda in0, in1, s0, s1, imm2: (
                ((in0 - (in0 >= np.float32(s1)).astype(np.float32))
                 .astype(np.float32) * np.float32(s0) + in1).astype(np.float32)
            ),
        ),
        subdim=False,
        uops_sha={"v3": "5dffcaa405b6c09a", "v4": "7706b30f0e4fb094"},
    )
    dve_ops.OPS.append(op)
    dve_ops.CUSTOM_DVE_SPECS[op.name] = op.spec
    dve_ops._SUB_OPCODE_FOR_NAME[op.name] = (
        dve_ops._CUSTOM_DVE_ROW_BASE + len(dve_ops.OPS) - 1
    )
    return op


def _build(repeat: int = 1):
    lif_u = _get_lif_op()
    nc = bacc.Bacc(
        "TRN2", target_bir_lowering=False, debug=False, num_devices=N_CORES
    )
    x = nc.dram_tensor(
        "x", [P, T * FB], mybir.dt.float32, kind="ExternalInput"
    ).ap()
    s_out = nc.dram_tensor(
        "s", [P, 2 * FB], mybir.dt.uint8, kind="ExternalOutput"
    ).ap()

    with tile.TileContext(nc) as tc:
        _emit(nc, tc, x, s_out, repeat, lif_u)

    nc.compile()
    return nc


def _emit(nc, tc, x, s_out, repeat, lif_u):
    Sign = mybir.ActivationFunctionType.Sign
    Copy = mybir.ActivationFunctionType.Copy
    h = FB // 2

    def xsl(t, a=0, b=FB):
        return slice(t * FB + a, t * FB + b)

    with ExitStack() as ctx:
        xp = ctx.enter_context(tc.tile_pool(name="xp", bufs=1))
        up = ctx.enter_context(tc.tile_pool(name="up", bufs=3))
        sp = ctx.enter_context(tc.tile_pool(name="sp", bufs=3))
        wp = ctx.enter_context(tc.tile_pool(name="wp", bufs=2))
        op = ctx.enter_context(tc.tile_pool(name="op", bufs=1))
        pp = ctx.enter_context(tc.tile_pool(name="pp", bufs=1, space="PSUM"))

        qs = [nc.sync, nc.scalar]

        for _ in range(repeat):
            xall = xp.tile([P, T * FB], mybir.dt.float32)

            # --- input loads: both HWDGE queues, emission order == queue
            # order. Full-timestep transfers except the tail (t14 halves,
            # t15 quarters) so the final bytes land as early as possible.
            for t in range(14):
                qs[t % 2].dma_start(xall[:, xsl(t)], x[:, xsl(t)])
            qs[0].dma_start(xall[:, xsl(14, 0, h)], x[:, xsl(14, 0, h)])
            qs[1].dma_start(xall[:, xsl(14, h, FB)], x[:, xsl(14, h, FB)])
            for k in range(4):
                qs[k % 2].dma_start(
                    xall[:, xsl(15, k * Q, (k + 1) * Q)],
                    x[:, xsl(15, k * Q, (k + 1) * Q)],
                )

            # --- pack weights on gpsimd (idle engine): 8 scaled identity
            # blocks w[:, j*128:(j+1)*128] = 2^j * I in bf16.
            vals = wp.tile([P, 8 * 128], mybir.dt.bfloat16, bufs=1)
            w = wp.tile([P, 8 * 128], mybir.dt.bfloat16, bufs=1)
            for j in range(8):
                nc.gpsimd.memset(vals[:, j * 128:(j + 1) * 128], float(2 ** j))
            nc.gpsimd.affine_select(
                w, vals, pattern=[[0, 8], [-1, 128]], base=0,
                channel_multiplier=1,
                compare_op=mybir.AluOpType.is_equal, fill=0.0,
            )

            psum = pp.tile([P, 2 * FB], mybir.dt.float32)   # all 8 banks
            outp = op.tile([P, 2 * FB], mybir.dt.uint8)

            def lif(out_ap, in0_ap, in1_ap):
                nc.vector._custom_dve(
                    lif_u, out=out_ap, in0=in0_ap, in1=in1_ap,
                    s0=BETA, s1=V_TH,
                )

            def sgn(out_ap, in_ap):
                # sign(1 - u) = -sign(u - 1): +1/0/-1 in bf16 (exact).
                # (bias must be a pre-registered const AP; 1.0 exists,
                # -1.0 doesn't -- host decode inverts the packed bits.)
                nc.scalar.activation(out_ap, in_ap, Sign, bias=V_TH, scale=-1.0)

            def mm(t, s_t, a, b):
                g, j = t // 8, t % 8
                for qa in range(a, b, Q):
                    nc.tensor.matmul(
                        psum[:, g * FB + qa: g * FB + qa + Q],
                        lhsT=w[:, j * 128:(j + 1) * 128],
                        rhs=s_t[:, qa:qa + Q],
                        start=(j == 0), stop=(j == 7),
                    )

            def pack(a, b):
                # packed byte = 0.5 * psum + 127.5 (exact integer in f32)
                nc.scalar.activation(
                    outp[:, a:b], psum[:, a:b], Copy, bias=127.5, scale=0.5,
                )

            # t0: u0 = x0, no LIF op
            s0 = sp.tile([P, FB], mybir.dt.bfloat16, name="s", tag="s")
            sgn(s0[:], xall[:, xsl(0)])
            mm(0, s0[:], 0, FB)
            u_prev = xall[:, xsl(0)]

            for t in range(1, 14):
                un = up.tile([P, FB], mybir.dt.float32, name="u", tag="u")
                lif(un[:], u_prev, xall[:, xsl(t)])
                st = sp.tile([P, FB], mybir.dt.bfloat16, name="s", tag="s")
                sgn(st[:], un[:])
                mm(t, st[:], 0, FB)
                u_prev = un[:]
                if t == 7:
                    # plane A complete: pack + SWDGE store (hidden under
                    # the ongoing loads)
                    pack(0, FB)
                    nc.gpsimd.dma_start(s_out[:, 0:FB], outp[:, 0:FB])

            # t14 in halves (matches its load granularity)
            u14 = up.tile([P, FB], mybir.dt.float32, name="u", tag="u")
            s14 = sp.tile([P, FB], mybir.dt.bfloat16, name="s", tag="s")
            for half in (0, 1):
                sl = slice(half * h, (half + 1) * h)
                lif(u14[:, sl], u_prev[:, sl], xall[:, xsl(14, sl.start, sl.stop)])
                sgn(s14[:, sl], u14[:, sl])
                mm(14, s14[:], sl.start, sl.stop)

            # t15 in quarters: lif -> sign -> matmul(stop) -> pack -> store,
            # stores on the by-now-idle HWDGE load queues
            u15 = up.tile([P, FB], mybir.dt.float32, name="u", tag="u")
            s15 = sp.tile([P, FB], mybir.dt.bfloat16, name="s", tag="s")
            for k in range(4):
                sl = slice(k * Q, (k + 1) * Q)
                lif(u15[:, sl], u14[:, sl], xall[:, xsl(15, sl.start, sl.stop)])
                sgn(s15[:, sl], u15[:, sl])
                mm(15, s15[:], sl.start, sl.stop)
                pack(FB + sl.start, FB + sl.stop)
                qs[k % 2].dma_start(
                    s_out[:, FB + sl.start:FB + sl.stop],
                    outp[:, FB + sl.start:FB + sl.stop],
                )


def _get_nc(repeat: int = 1):
    key = f"nc{repeat}"
    if key not in _CACHE:
        _CACHE[key] = _build(repeat)
    return _CACHE[key]


def _shard_input(x_seq: np.ndarray, i: int) -> np.ndarray:
    # [4, T, C, H, W] -> partition-major arena layout [P, T*B_LOC*F].
    xc = x_seq[i * B_LOC:(i + 1) * B_LOC].reshape(B_LOC, T, P, F)
    return np.ascontiguousarray(xc.transpose(2, 1, 0, 3).reshape(P, T * FB))


def _unshard_output(s_u8: np.ndarray) -> np.ndarray:
    # [P, 2*FB] u8 -> [B_LOC, T, C, H, W] f32 spikes.
    # Plane g in {0,1} holds bit j = spike at t = 8g + j, inverted
    # (on-chip sign is sign(1-u), so stored byte = 255 - packed bits).
    pl = (~s_u8).reshape(P, 2, B_LOC, F)
    bits = np.unpackbits(pl[..., None], axis=-1, bitorder="little")
    # dims (P, g, b, F, j) -> (b, g, j, P, F) -> [B_LOC, T, P, F]
    s = bits.transpose(2, 1, 4, 0, 3).reshape(B_LOC, T, P, F)
    return s.astype(np.float32).reshape(B_LOC, T, C, H, W)


def _run(x_seq: np.ndarray, trace: bool = False, repeat: int = 1):
    """Shard, execute on 8 cores, gather. Returns (output, BassKernelResults)."""
    nc = _get_nc(repeat)
    x_seq = np.ascontiguousarray(x_seq, dtype=np.float32)
    in_maps = [{"x": _shard_input(x_seq, i)} for i in range(N_CORES)]
    res = run_bass_kernel_spmd(
        nc, in_maps, core_ids=list(range(N_CORES)), trace=trace
    )
    out = np.concatenate(
        [_unshard_output(r["s"]) for r in res.results], axis=0
    )
    return out, res


def kernel(x_seq: np.ndarray) -> np.ndarray:
    out, _ = _run(x_seq, trace=False)
    return out
